# revision 32
# baseline (speedup 1.0000x reference)
"""BFP-quantized 3x3 conv (nn_BFConv2d) on 8 TRN2 NeuronCores.

Strategy (data-parallel over batch, 4 samples/core):
  Program A (quantize): per core, for each of its 4 samples, load a
    group-aligned window of the flattened x (the BFP group grid is global
    over the flat tensor; each per-sample window starts on a 36-element
    group boundary, so the in-kernel grid is exact), compute the BFP
    quantization with the magic-number trick
        q = (x + M) - M,  M = 1.5 * 2^23 * scale = exp_bits(absmax) * 98304
    (exact round-half-even onto the group lattice; results are <=9
    significant bits so bf16 is exact), and write q as bf16. The weight
    tensor (36864 elems = exactly 1024 groups) is quantized the same way.
  Host: slice each sample's quantized window by its group-grid phase
    (pre in [0,36)) to get slab-aligned q; pure numpy, no device work.
  Program B (conv): 3x3 conv as 9 shifted 64x64 bf16 matmuls per output
    tile, using TensorE 64x64 array tiling: quadrant (0,0) processes the
    even sample of a pair (SBUF partitions 0-63), quadrant (64,64) the odd
    sample (partitions 64-127), both accumulating into one PSUM bank.
    ScalarE evacuates PSUM with the bias add fused; one full-width DMA
    writes both samples' rows (64*12544 == 802816 makes the pair layout
    contiguous in NCHW).
"""

import os
import sys
from contextlib import ExitStack

import numpy as np

sys.path.insert(0, "/opt/trn_rl_repo")

import ml_dtypes  # noqa: E402
import concourse.bacc as bacc  # noqa: E402
import concourse.bass as bass  # noqa: E402
import concourse.mybir as mybir  # noqa: E402
import concourse.tile as tile  # noqa: E402

F32 = mybir.dt.float32
BF16 = mybir.dt.bfloat16
I32 = mybir.dt.int32

N_CORES = 8
B = 32                      # batch
C = 64                      # channels (in == out)
H = W = 112
SAMPLE = C * H * W          # 802816 elems per sample
GS = 36                     # BFP group size
GPP = 175                   # groups per partition in the quantize window
QCOLS = GPP * GS            # 6300
QWIN = 128 * QCOLS          # 806400 elems: covers a sample + phase slack
WP = W + 2                  # padded row width 114
XPAD = WP * WP + 2          # padded sample + 2 guard slots
MAGIC_MUL = 98304.0         # 1.5 * 2^16:  exp2(e) * this == 1.5*2^23*2^(e-7)

_cache = {}
last_exec_ns = {}
last_results = {}


def _ensure_snap_op():
    """Register a custom DVE op BFP_SNAP_ANT: out = (in0 + in1) - in1.

    One streaming pass for the BFP magic-number snap (vs add + subtract as
    two scalar_tensor_tensor passes). The per-NEFF DVE table machinery picks
    it up from dve_ops.OPS; sha is pinned from this environment's lowering.
    """
    import concourse.dve_ops as dops
    if getattr(dops, "_BFP_SNAP_ANT", None) is not None:
        return dops._BFP_SNAP_ANT
    from concourse.dve_spec import Spec, Src0, Src1, lower as spec_lower
    from concourse.dve_uop import DveOpSpec

    def _snap_ref(in0, in1, s0, s1, imm2):
        a = in0.astype(np.float32)
        b = np.broadcast_to(in1.astype(np.float32), in1.shape).reshape(a.shape)
        return (a + b) - b

    spec = Spec(body=(Src0 + Src1) - Src1, reference=_snap_ref)
    op = dops.DveOp("BFP_SNAP_ANT", spec, subdim=False, uops_sha={})
    idx = max(dops._SUB_OPCODE_FOR_NAME.values()) + 1
    assert idx < 0x20
    dops.OPS.append(op)
    dops.CUSTOM_DVE_SPECS["BFP_SNAP_ANT"] = spec
    dops._SUB_OPCODE_FOR_NAME["BFP_SNAP_ANT"] = idx
    for ver in ("v3", "v4"):
        try:
            s = DveOpSpec(name=op.name, opcode=idx,
                          uops=spec_lower(spec, ver=ver), rd1_en=True)
            op.uops_sha[ver] = s.sha(ver)
        except Exception:
            pass
    dops._BFP_SNAP_ANT = op
    return op


def _trace_enabled():
    return os.environ.get("BFP_TRACE") == "1"


def _install_trace_shim():
    """Provide antenv.axon_hooks (NTFF profiling hook) if the image lacks it.

    Mirrors trn_agent_boot.trn_boot._ntff_profile_via_ctypes: drives NRT
    profiling through the axon PJRT .so so run_bass_kernel_spmd(trace=True)
    can report HW exec time.
    """
    import types
    import ctypes
    import contextlib
    try:
        from antenv.axon_hooks import get_axon_ntff_profile_hook  # noqa: F401
        return
    except ImportError:
        pass
    so_path = "/opt/axon/libaxon_pjrt.so"
    if not os.path.exists(so_path):
        return
    lib = ctypes.CDLL(so_path)
    if not hasattr(lib, "axon_start_nrt_profile"):
        return
    lib.axon_start_nrt_profile.argtypes = [ctypes.POINTER(ctypes.c_int64),
                                           ctypes.c_size_t]
    lib.axon_start_nrt_profile.restype = ctypes.c_int64
    lib.axon_stop_nrt_profile.argtypes = [ctypes.c_char_p]
    lib.axon_stop_nrt_profile.restype = ctypes.c_int64

    @contextlib.contextmanager
    def _hook(output_dir, device_ids):
        import jax
        jax.devices()
        if device_ids:
            ids = (ctypes.c_int64 * len(device_ids))(*device_ids)
            rc = lib.axon_start_nrt_profile(ids, len(device_ids))
        else:
            rc = lib.axon_start_nrt_profile(None, 0)
        if rc != 0:
            raise RuntimeError(f"axon_start_nrt_profile rc={rc}")
        try:
            yield
        finally:
            n = lib.axon_stop_nrt_profile(str(output_dir).encode())
            print(f"profile: {n} ntff file(s) -> {output_dir}", file=sys.stderr)

    mod = types.ModuleType("antenv.axon_hooks")
    state = {"hook": _hook}
    mod.get_axon_ntff_profile_hook = lambda: state["hook"]
    mod.set_axon_ntff_profile_hook = lambda h: state.update(hook=h)
    sys.modules["antenv.axon_hooks"] = mod
    import antenv
    antenv.axon_hooks = mod
    from concourse import bass_utils as bu
    bu.upload_artifacts = lambda d: str(d)  # no egress from this container


I16 = mybir.dt.int16
# free-dim split of a sample window into 2 group-aligned chunks
CHUNK_COLS = (3168, 3132)          # 88 + 87 groups = 175
CHUNK_OFF = (0, 3168)


def build_quant():
    """v2: scalar casts f32->bf16; vector reduces bf16 at 2x and snaps;
    gpsimd computes per-group magic constants. Half-sample chunks."""
    snap = _ensure_snap_op()
    nc = bacc.Bacc(None)
    xin = nc.declare_dram_parameter("xin", [4, 128, QCOLS], F32, isOutput=False)
    win = nc.declare_dram_parameter("w", [C, C, 3, 3], F32, isOutput=False)
    qx = nc.declare_dram_parameter("qx", [4, 128, QCOLS], BF16, isOutput=True)
    qw = nc.declare_dram_parameter("qw", [128, 288], BF16, isOutput=True)

    def bfp_w(pool, spool, src_ap, ngroups, out_tile):
        """f32 path for the small weight tensor (exact, as v1)."""
        g3 = lambda ap: ap.rearrange("p (g s) -> p g s", s=GS)
        m = spool.tile([128, ngroups], F32, tag="wm")
        nc.vector.tensor_reduce(m[:], g3(src_ap), axis=mybir.AxisListType.X,
                                op=mybir.AluOpType.max, apply_absolute_value=True)
        mi = spool.tile([128, ngroups], I32, tag="wmi")
        nc.vector.tensor_scalar(mi[:], m[:].bitcast(I32), 0x7F800000, None,
                                op0=mybir.AluOpType.bitwise_and)
        mf = spool.tile([128, ngroups], F32, tag="wmf")
        nc.vector.tensor_scalar(mf[:], mi[:].bitcast(F32), MAGIC_MUL, None,
                                op0=mybir.AluOpType.mult)
        mb = mf[:].unsqueeze(-1).broadcast_to([128, ngroups, GS])
        nc.vector._custom_dve(snap, out=g3(out_tile[:]), in0=g3(src_ap), in1=mb)

    with tile.TileContext(nc) as tc:
        with ExitStack() as ctx:
            pool = ctx.enter_context(tc.tile_pool(name="big", bufs=3))
            spool = ctx.enter_context(tc.tile_pool(name="small", bufs=3))
            # weight first: its tiny DMA lands long before sample 0's slab
            wf = pool.tile([128, 288], F32, tag="wf")
            nc.sync.dma_start(wf[:], win[:].rearrange("o i h w -> (o i h w)")
                              .rearrange("(p c) -> p c", p=128))
            qwt = pool.tile([128, 288], BF16, tag="qwt")
            bfp_w(pool, spool, wf[:], 8, qwt)
            nc.scalar.dma_start(qw[:], qwt[:])
            xr = xin[:].rearrange("j p c -> p j c")
            qr = qx[:].rearrange("j p c -> p j c")
            for j in range(4):
                for h in range(2):
                    cols, off = CHUNK_COLS[h], CHUNK_OFF[h]
                    ng = cols // GS
                    g3 = lambda ap: ap.rearrange("p (g s) -> p g s", s=GS)
                    xs = pool.tile([128, cols], F32, tag=f"xs{h}")
                    nc.sync.dma_start(xs[:], xr[:, j, off:off + cols])
                    # scalar engine: cast to bf16 (feeds both reduce and snap)
                    xb = pool.tile([128, cols], BF16, tag=f"xb{h}")
                    nc.scalar.copy(xb[:], xs[:])
                    # vector: group abs-max on bf16 (2x mode)
                    m = spool.tile([128, ng], BF16, tag=f"m{h}")
                    nc.vector.tensor_reduce(
                        m[:], g3(xb[:]), axis=mybir.AxisListType.X,
                        op=mybir.AluOpType.max, apply_absolute_value=True)
                    # M = exp_bits(max) * 1.5*2^16  (tiny)
                    mi = spool.tile([128, ng], I16, tag=f"mi{h}")
                    nc.vector.tensor_scalar(
                        mi[:], m[:].bitcast(I16), 0x7F80, None,
                        op0=mybir.AluOpType.bitwise_and)
                    mf = spool.tile([128, ng], BF16, tag=f"mf{h}")
                    nc.vector.tensor_scalar(
                        mf[:], mi[:].bitcast(BF16), MAGIC_MUL, None,
                        op0=mybir.AluOpType.mult)
                    # vector: snap (x + M) - M in one custom-DVE pass
                    q = pool.tile([128, cols], BF16, tag=f"q{h}", bufs=6)
                    mb = mf[:].unsqueeze(-1).broadcast_to([128, ng, GS])
                    nc.vector._custom_dve(snap, out=g3(q[:]), in0=g3(xb[:]),
                                          in1=mb)
                    nc.scalar.dma_start(qr[:, j, off:off + cols], q[:])
    nc.compile()
    return nc


def build_conv():
    """v2: 4x 64x64 PE-array tiles. T0/T2 convolve sample A (row-chunks 2t,
    2t+1), T8/T10 sample B, all four concurrently; weights replicated on
    both SBUF partition halves. Two PSUM banks per round (A and B)."""
    nc = bacc.Bacc(None)
    qx4 = nc.declare_dram_parameter("qx4", [4, C, WP, WP], BF16, isOutput=False)
    wblk = nc.declare_dram_parameter("wblk", [128, 9 * 64], BF16, isOutput=False)
    bias2 = nc.declare_dram_parameter("bias2", [128], F32, isOutput=False)
    # chunk-major: [sample, 4-row chunk, c, r, w]; host transposes to NCHW.
    # Keeps each out-DMA one call with 128 contiguous 1792B descriptors.
    out = nc.declare_dram_parameter("out", [4, 2, C, 14, 4, W], F32,
                                    isOutput=True)

    with tile.TileContext(nc) as tc:
        with ExitStack() as ctx:
            consts = ctx.enter_context(tc.tile_pool(name="consts", bufs=1))
            xpool = ctx.enter_context(tc.tile_pool(name="x", bufs=2))
            opool = ctx.enter_context(tc.tile_pool(name="o", bufs=4))
            psum = ctx.enter_context(tc.tile_pool(name="ps", bufs=2, space="PSUM"))

            # wblk[p, tap*64 + o]: W[tap][ic=p%64, oc=o], same on both halves
            wsb = consts.tile([128, 9 * 64], BF16)
            nc.sync.dma_start(wsb[:], wblk[:])
            bias_sb = consts.tile([128, 1], F32)
            nc.sync.dma_start(bias_sb[:], bias2[:, None])

            for p in range(2):
                xpad = xpool.tile([128, XPAD], BF16, tag="xpad")
                nc.gpsimd.memset(xpad[:, 0:1], 0.0)           # guard slots
                nc.gpsimd.memset(xpad[:, XPAD - 1:XPAD], 0.0)
                # load in 4 row-bands so round 0 can start after band 0;
                # band k covers padded rows [29k, 29k+29) (+1 trailing row)
                for k in range(4):
                    rlo = 29 * k
                    nrows = 29 if k < 3 else 27
                    nc.sync.dma_start(
                        xpad[0:64, 1 + rlo * WP:1 + (rlo + nrows) * WP],
                        qx4[2 * p, :, rlo:rlo + nrows, :]
                        .rearrange("c h w -> c (h w)"))
                    nc.sync.dma_start(
                        xpad[64:128, 1 + rlo * WP:1 + (rlo + nrows) * WP],
                        qx4[2 * p + 1, :, rlo:rlo + nrows, :]
                        .rearrange("c h w -> c (h w)"))

                for t in range(14):
                    r0 = 8 * t
                    psA = psum.tile([128, 456], F32, tag="psA")
                    psB = psum.tile([128, 456], F32, tag="psB")
                    for tap in range(9):
                        dh, dw = divmod(tap, 3)
                        b0 = 1 + (r0 + dh) * WP + dw - 1
                        b1 = 1 + (r0 + 4 + dh) * WP + dw - 1
                        st, sp = (tap == 0), (tap == 8)
                        w_lo = wsb2[0:64, tap * 64:(tap + 1) * 64]
                        w_hi = wsb2[64:128, tap * 64:(tap + 1) * 64]
                        nc.tensor.matmul(psA[0:64, :], w_lo,
                                         xpad[0:64, b0:b0 + 456],
                                         start=st, stop=sp,
                                         tile_position=(0, 0))
                        nc.tensor.matmul(psB[0:64, :], w_hi,
                                         xpad[64:128, b0:b0 + 456],
                                         start=st, stop=sp,
                                         tile_position=(64, 0))
                        nc.tensor.matmul(psA[64:128, :], w_lo,
                                         xpad[0:64, b1:b1 + 456],
                                         start=st, stop=sp,
                                         tile_position=(0, 64))
                        nc.tensor.matmul(psB[64:128, :], w_hi,
                                         xpad[64:128, b1:b1 + 456],
                                         start=st, stop=sp,
                                         tile_position=(64, 64))
                    for s, ps in ((0, psA), (1, psB)):
                        # compact 114->112 in the evac so the DMA source is
                        # contiguous: one 1792B descriptor per partition
                        osb = opool.tile([128, 448], F32, tag=f"osb{s}")
                        nc.vector.tensor_scalar(
                            osb[:].rearrange("p (r w) -> p r w", w=W),
                            ps[:].rearrange("p (r w) -> p r w", w=WP)
                            [:, :, 1:113],
                            bias_sb[:, 0:1], None,
                            op0=mybir.AluOpType.add)
                        nc.scalar.dma_start(
                            out[2 * p + s, 2 * t:2 * t + 2]
                            .rearrange("u c r w -> (u c) (r w)"),
                            osb[:])
    nc.compile()
    return nc


def build_fused():
    """Single program: BFP-quantize x (f32 path), bounce q through a DRAM
    scratch tile, indirect-DMA it back in conv layout (per-channel dynamic
    offsets absorb the per-sample group phase), then the 4x64x64-tiled conv.
    Weights arrive pre-quantized from the host (exact numpy BFP)."""
    snap = _ensure_snap_op()
    nc = bacc.Bacc(None, num_swdge_queues=2)
    xin = nc.declare_dram_parameter("xin", [4, 128, QCOLS], F32, isOutput=False)
    wblk = nc.declare_dram_parameter("wblk", [128, 9 * 64], BF16, isOutput=False)
    bias2 = nc.declare_dram_parameter("bias2", [128], F32, isOutput=False)
    ofs = nc.declare_dram_parameter("ofs", [32, 8], I32, isOutput=False)
    out = nc.declare_dram_parameter("out", [4, 2, C, 14, 4, W], F32,
                                    isOutput=True)

    g3 = lambda ap: ap.rearrange("p (g s) -> p g s", s=GS)

    with tile.TileContext(nc) as tc:
        with ExitStack() as ctx:
            consts = ctx.enter_context(tc.tile_pool(name="consts", bufs=1))
            qpool = ctx.enter_context(tc.tile_pool(name="qd", bufs=1,
                                                   space="DRAM"))
            xpool = ctx.enter_context(tc.tile_pool(name="x", bufs=2))
            spool = ctx.enter_context(tc.tile_pool(name="small", bufs=3))
            xpadp = ctx.enter_context(tc.tile_pool(name="xp", bufs=2))
            opool = ctx.enter_context(tc.tile_pool(name="o", bufs=4))
            psum = ctx.enter_context(tc.tile_pool(name="ps", bufs=4,
                                                  space="PSUM"))

            wsb = consts.tile([128, 9 * 64], BF16)
            nc.sync.dma_start(wsb[:], wblk[:])
            bias_sb = consts.tile([128, 1], F32)
            nc.sync.dma_start(bias_sb[:], bias2[:, None])
            ofs_sb = consts.tile([32, 8], I32)
            nc.sync.dma_start(ofs_sb[:], ofs[:])

            qfs = [qpool.tile([128 * QCOLS, 1], BF16, tag=f"qf{j}",
                              name=f"qf{j}") for j in range(4)]

            def quant_sample(j):
                qlast = None
                for h in range(2):
                    cols, off = CHUNK_COLS[h], CHUNK_OFF[h]
                    ng = cols // GS
                    xs = xpool.tile([128, cols], F32, tag=f"xs{h}")
                    nc.sync.dma_start(xs[:],
                                      xin[:].rearrange("j p c -> p j c")
                                      [:, j, off:off + cols])
                    m = spool.tile([128, ng], F32, tag=f"m{h}")
                    nc.vector.tensor_reduce(
                        m[:], g3(xs[:]), axis=mybir.AxisListType.X,
                        op=mybir.AluOpType.max, apply_absolute_value=True)
                    mi = spool.tile([128, ng], I32, tag=f"mi{h}")
                    nc.vector.tensor_scalar(
                        mi[:], m[:].bitcast(I32), 0x7F800000, None,
                        op0=mybir.AluOpType.bitwise_and)
                    mf = spool.tile([128, ng], F32, tag=f"mf{h}")
                    nc.vector.tensor_scalar(
                        mf[:], mi[:].bitcast(F32), MAGIC_MUL, None,
                        op0=mybir.AluOpType.mult)
                    q = xpool.tile([128, cols], BF16, tag=f"q{h}", bufs=2)
                    mb = mf[:].unsqueeze(-1).broadcast_to([128, ng, GS])
                    nc.vector._custom_dve(snap, out=g3(q[:]), in0=g3(xs[:]),
                                          in1=mb)
                    nc.scalar.dma_start(
                        qfs[j][:].rearrange("(p c) one -> p (c one)", p=128)
                        [:, off:off + cols], q[:])
                    qlast = q
                return qlast

            def readback(j, xflat):
                half = j % 2
                pb = 64 * half
                if os.environ.get("FV_STATIC_RB"):
                    nc.sync.dma_start(
                        xflat[pb:pb + 64, :],
                        qfs[j][:].rearrange("n one -> (n one)")
                        [0:64 * 12544].rearrange("(c f) -> c f", f=12544))
                    return
                for piece in range(2):
                    c0 = 32 * piece
                    nc.gpsimd.indirect_dma_start(
                        out=xflat[pb + c0:pb + c0 + 32, :],
                        out_offset=None,
                        in_=qfs[j][:],
                        in_offset=bass.IndirectOffsetOnAxis(
                            ap=ofs_sb[0:32, 2 * j + piece:2 * j + piece + 1],
                            axis=0))

            # tap validity: out(r,w) += W[dh,dw] * x(r+dh-1, w+dw-1); taps
            # reaching outside the sample are simply dropped (zero padding)
            TAPS = [(1, 1)] + [(dh, dw) for dh in range(3) for dw in range(3)
                               if (dh, dw) != (1, 1)]
            if os.environ.get("FV_CENTER_ONLY"):
                TAPS = [(1, 1)] * 9

            def conv_pair(p, xflat, wsb2=None):
                xv = xflat[:].rearrange("p (r w) -> p r w", w=W)
                stg = [opool.tile([128, 14 * 448], F32, tag=f"stg{s}",
                                  bufs=1, name=f"stg{s}_{p}")
                       for s in range(2)]
                for t in range(14):
                    r0 = 8 * t
                    psA = psum.tile([128, 448], F32, tag="psA")
                    psB = psum.tile([128, 448], F32, tag="psB")
                    views = {id(psA): psA[:].rearrange("p (r w) -> p r w", w=W),
                             id(psB): psB[:].rearrange("p (r w) -> p r w", w=W)}
                    for n, (dh, dw) in enumerate(TAPS):
                        st, sp = (n == 0), (n == 8)
                        tap = dh * 3 + dw
                        w_lo = wsb2[0:64, tap * 64:(tap + 1) * 64]
                        w_hi = wsb2[64:128, tap * 64:(tap + 1) * 64]
                        wlo = max(0, 1 - dw)
                        whi = min(W, W + 1 - dw)
                        mw = wlo + dw - 1
                        for half, (ps, wq, kb, cq) in enumerate((
                                (psA, w_lo, 0, 0), (psB, w_hi, 64, 0),
                                (psA, w_lo, 0, 64), (psB, w_hi, 64, 64))):
                            R = r0 + 4 * (cq // 64)
                            ilo = max(0, 1 - dh - R)
                            ihi = min(4, H + 1 - dh - R)
                            if ilo >= ihi or wlo >= whi:
                                continue
                            rb = R + ilo + dh - 1
                            pv = views[id(ps)]
                            nc.tensor.matmul(
                                pv[cq:cq + 64, ilo:ihi, wlo:whi],
                                wq,
                                xv[kb:kb + 64, rb:rb + ihi - ilo, mw:mw + whi - wlo],
                                start=st, stop=sp,
                                tile_position=(kb, cq))
                    for s, ps in ((0, psA), (1, psB)):
                        nc.scalar.add(stg[s][:, 448 * t:448 * t + 448],
                                      ps[:], bias_sb[:, 0:1])
                for s in range(2):
                    for hh in range(4):
                        t0 = (0, 4, 8, 11)[hh]
                        t1 = (4, 8, 11, 14)[hh]
                        nc.scalar.dma_start(
                            out[2 * p + s, :, :, t0:t1]
                            .rearrange("u c t r w -> (u c) (t r w)"),
                            stg[s][:, 448 * t0:448 * t1])

            xflats = {}
            for p in range(2):
                xflats[p] = xpadp.tile([128, SAMPLE // 64], BF16, tag="xpad",
                                       name=f"xflat{p}")
                for j in (2 * p, 2 * p + 1):
                    qlast = quant_sample(j)
                    readback(j, xflats[p])
            # value-safe dependency: conv phase starts only after the last
            # readback, so the tensor engine runs with the DMA/vector quiet
            # (avoids the power-state penalty measured when phases overlap)
            tok = consts.tile([128, 1], F32, name="tok")
            nc.vector.tensor_tensor(tok[:], qlast[:, 0:1], qlast[:, 0:1],
                                    op=mybir.AluOpType.is_equal)
            wsb2 = consts.tile([128, 9 * 64], BF16, name="wsb2")
            nc.vector.tensor_scalar(wsb2[:], wsb[:], tok[:, 0:1], None,
                                    op0=mybir.AluOpType.mult)
            for p in range(2):
                conv_pair(p, xflats[p], wsb2)
    nc.compile()
    return nc


def _bfp_quantize_np(w):
    """Exact host-side BFP quantization (matches the jax reference bit-for-
    bit in f32: scale is a power of two, round is half-even)."""
    flat = np.asarray(w, np.float32).reshape(-1)
    n = flat.shape[0]
    pad = (-n) % GS
    f = np.pad(flat, (0, pad)).reshape(-1, GS)
    m = np.max(np.abs(f), axis=1, keepdims=True)
    mant, ex = np.frexp(np.where(m > 0, m, 1.0))   # m = mant*2^ex, mant in [.5,1)
    scale = np.ldexp(np.float32(1.0), ex - 1 - 7).astype(np.float32)
    q = np.round(f / scale) * scale
    q = np.where(m > 0, q, 0.0).astype(np.float32)
    return q.reshape(-1)[:n].reshape(np.asarray(w).shape)


def _shard_inputs(x, weight):
    """Build per-core in_maps for program A."""
    xf = np.ascontiguousarray(x, dtype=np.float32).reshape(-1)
    xf = np.concatenate([xf, np.zeros(QWIN, np.float32)])
    in_maps = []
    pres = []
    for k in range(N_CORES):
        core_pre = []
        xin = np.empty((4, 128, QCOLS), np.float32)
        for j in range(4):
            s = 4 * k + j
            start = s * SAMPLE
            gstart = (start // GS) * GS
            core_pre.append(start - gstart)
            xin[j] = xf[gstart:gstart + QWIN].reshape(128, QCOLS)
        in_maps.append({"xin": xin, "w": np.ascontiguousarray(weight, np.float32)})
        pres.append(core_pre)
    return in_maps, pres


def kernel(x, weight, bias):
    from concourse.bass_utils import run_bass_kernel_spmd

    if "fused" not in _cache:
        _cache["fused"] = build_fused()

    core_ids = list(range(N_CORES))
    trace = _trace_enabled()
    if trace:
        _install_trace_shim()

    in_maps, pres = _shard_inputs(x, weight)

    wq = _bfp_quantize_np(weight).reshape(64, 64, 9)    # [o,i,t] exact BFP
    wtio = wq.transpose(1, 2, 0).astype(ml_dtypes.bfloat16)  # [i,t,o]
    wblk = np.concatenate([wtio, wtio], axis=0).reshape(128, 9 * 64)
    bias2 = np.concatenate([np.asarray(bias, np.float32)] * 2)
    rvec = np.arange(32, dtype=np.int64) * (H * W)
    for k in range(N_CORES):
        ofs = np.empty((32, 8), np.int64)
        for j in range(4):
            for piece in range(2):
                ofs[:, 2 * j + piece] = (pres[k][j] + (32 * piece) * (H * W)
                                         + rvec)
        in_maps[k] = {"xin": in_maps[k]["xin"], "wblk": wblk,
                      "bias2": bias2, "ofs": ofs.astype(np.int32)}

    res = run_bass_kernel_spmd(_cache["fused"], in_maps, core_ids, trace=trace)
    last_exec_ns.clear()
    last_exec_ns["fused"] = res.exec_time_ns
    last_results["fused"] = res

    out = np.concatenate(
        [np.asarray(res.results[k]["out"]) for k in range(N_CORES)], axis=0)
    # [32, 2, C, 14, 4, W] -> NCHW (h = 8t + 4u + r)
    out = out.transpose(0, 2, 3, 1, 4, 5).reshape(B, C, H, W)
    return np.ascontiguousarray(out, dtype=np.float32)



# revision 33
# speedup vs baseline: 1.1949x; 1.1949x over previous
"""BFP-quantized 3x3 conv (nn_BFConv2d) on 8 TRN2 NeuronCores.

Strategy (data-parallel over batch, 4 samples/core):
  Program A (quantize): per core, for each of its 4 samples, load a
    group-aligned window of the flattened x (the BFP group grid is global
    over the flat tensor; each per-sample window starts on a 36-element
    group boundary, so the in-kernel grid is exact), compute the BFP
    quantization with the magic-number trick
        q = (x + M) - M,  M = 1.5 * 2^23 * scale = exp_bits(absmax) * 98304
    (exact round-half-even onto the group lattice; results are <=9
    significant bits so bf16 is exact), and write q as bf16. The weight
    tensor (36864 elems = exactly 1024 groups) is quantized the same way.
  Host: slice each sample's quantized window by its group-grid phase
    (pre in [0,36)) to get slab-aligned q; pure numpy, no device work.
  Program B (conv): 3x3 conv as 9 shifted 64x64 bf16 matmuls per output
    tile, using TensorE 64x64 array tiling: quadrant (0,0) processes the
    even sample of a pair (SBUF partitions 0-63), quadrant (64,64) the odd
    sample (partitions 64-127), both accumulating into one PSUM bank.
    ScalarE evacuates PSUM with the bias add fused; one full-width DMA
    writes both samples' rows (64*12544 == 802816 makes the pair layout
    contiguous in NCHW).
"""

import os
import sys
from contextlib import ExitStack

import numpy as np

sys.path.insert(0, "/opt/trn_rl_repo")

import ml_dtypes  # noqa: E402
import concourse.bacc as bacc  # noqa: E402
import concourse.bass as bass  # noqa: E402
import concourse.mybir as mybir  # noqa: E402
import concourse.tile as tile  # noqa: E402

F32 = mybir.dt.float32
BF16 = mybir.dt.bfloat16
I32 = mybir.dt.int32

N_CORES = 8
B = 32                      # batch
C = 64                      # channels (in == out)
H = W = 112
SAMPLE = C * H * W          # 802816 elems per sample
GS = 36                     # BFP group size
GPP = 175                   # groups per partition in the quantize window
QCOLS = GPP * GS            # 6300
QWIN = 128 * QCOLS          # 806400 elems: covers a sample + phase slack
WP = W + 2                  # padded row width 114
XPAD = WP * WP + 2          # padded sample + 2 guard slots
MAGIC_MUL = 98304.0         # 1.5 * 2^16:  exp2(e) * this == 1.5*2^23*2^(e-7)

_cache = {}
last_exec_ns = {}
last_results = {}


def _ensure_snap_op():
    """Register a custom DVE op BFP_SNAP_ANT: out = (in0 + in1) - in1.

    One streaming pass for the BFP magic-number snap (vs add + subtract as
    two scalar_tensor_tensor passes). The per-NEFF DVE table machinery picks
    it up from dve_ops.OPS; sha is pinned from this environment's lowering.
    """
    import concourse.dve_ops as dops
    if getattr(dops, "_BFP_SNAP_ANT", None) is not None:
        return dops._BFP_SNAP_ANT
    from concourse.dve_spec import Spec, Src0, Src1, lower as spec_lower
    from concourse.dve_uop import DveOpSpec

    def _snap_ref(in0, in1, s0, s1, imm2):
        a = in0.astype(np.float32)
        b = np.broadcast_to(in1.astype(np.float32), in1.shape).reshape(a.shape)
        return (a + b) - b

    spec = Spec(body=(Src0 + Src1) - Src1, reference=_snap_ref)
    op = dops.DveOp("BFP_SNAP_ANT", spec, subdim=False, uops_sha={})
    idx = max(dops._SUB_OPCODE_FOR_NAME.values()) + 1
    assert idx < 0x20
    dops.OPS.append(op)
    dops.CUSTOM_DVE_SPECS["BFP_SNAP_ANT"] = spec
    dops._SUB_OPCODE_FOR_NAME["BFP_SNAP_ANT"] = idx
    for ver in ("v3", "v4"):
        try:
            s = DveOpSpec(name=op.name, opcode=idx,
                          uops=spec_lower(spec, ver=ver), rd1_en=True)
            op.uops_sha[ver] = s.sha(ver)
        except Exception:
            pass
    dops._BFP_SNAP_ANT = op
    return op


def _trace_enabled():
    return os.environ.get("BFP_TRACE") == "1"


def _install_trace_shim():
    """Provide antenv.axon_hooks (NTFF profiling hook) if the image lacks it.

    Mirrors trn_agent_boot.trn_boot._ntff_profile_via_ctypes: drives NRT
    profiling through the axon PJRT .so so run_bass_kernel_spmd(trace=True)
    can report HW exec time.
    """
    import types
    import ctypes
    import contextlib
    try:
        from antenv.axon_hooks import get_axon_ntff_profile_hook  # noqa: F401
        return
    except ImportError:
        pass
    so_path = "/opt/axon/libaxon_pjrt.so"
    if not os.path.exists(so_path):
        return
    lib = ctypes.CDLL(so_path)
    if not hasattr(lib, "axon_start_nrt_profile"):
        return
    lib.axon_start_nrt_profile.argtypes = [ctypes.POINTER(ctypes.c_int64),
                                           ctypes.c_size_t]
    lib.axon_start_nrt_profile.restype = ctypes.c_int64
    lib.axon_stop_nrt_profile.argtypes = [ctypes.c_char_p]
    lib.axon_stop_nrt_profile.restype = ctypes.c_int64

    @contextlib.contextmanager
    def _hook(output_dir, device_ids):
        import jax
        jax.devices()
        if device_ids:
            ids = (ctypes.c_int64 * len(device_ids))(*device_ids)
            rc = lib.axon_start_nrt_profile(ids, len(device_ids))
        else:
            rc = lib.axon_start_nrt_profile(None, 0)
        if rc != 0:
            raise RuntimeError(f"axon_start_nrt_profile rc={rc}")
        try:
            yield
        finally:
            n = lib.axon_stop_nrt_profile(str(output_dir).encode())
            print(f"profile: {n} ntff file(s) -> {output_dir}", file=sys.stderr)

    mod = types.ModuleType("antenv.axon_hooks")
    state = {"hook": _hook}
    mod.get_axon_ntff_profile_hook = lambda: state["hook"]
    mod.set_axon_ntff_profile_hook = lambda h: state.update(hook=h)
    sys.modules["antenv.axon_hooks"] = mod
    import antenv
    antenv.axon_hooks = mod
    from concourse import bass_utils as bu
    bu.upload_artifacts = lambda d: str(d)  # no egress from this container


I16 = mybir.dt.int16
# free-dim split of a sample window into 2 group-aligned chunks
CHUNK_COLS = (3168, 3132)          # 88 + 87 groups = 175
CHUNK_OFF = (0, 3168)


def build_quant():
    """v2: scalar casts f32->bf16; vector reduces bf16 at 2x and snaps;
    gpsimd computes per-group magic constants. Half-sample chunks."""
    snap = _ensure_snap_op()
    nc = bacc.Bacc(None)
    xin = nc.declare_dram_parameter("xin", [4, 128, QCOLS], F32, isOutput=False)
    win = nc.declare_dram_parameter("w", [C, C, 3, 3], F32, isOutput=False)
    qx = nc.declare_dram_parameter("qx", [4, 128, QCOLS], BF16, isOutput=True)
    qw = nc.declare_dram_parameter("qw", [128, 288], BF16, isOutput=True)

    def bfp_w(pool, spool, src_ap, ngroups, out_tile):
        """f32 path for the small weight tensor (exact, as v1)."""
        g3 = lambda ap: ap.rearrange("p (g s) -> p g s", s=GS)
        m = spool.tile([128, ngroups], F32, tag="wm")
        nc.vector.tensor_reduce(m[:], g3(src_ap), axis=mybir.AxisListType.X,
                                op=mybir.AluOpType.max, apply_absolute_value=True)
        mi = spool.tile([128, ngroups], I32, tag="wmi")
        nc.vector.tensor_scalar(mi[:], m[:].bitcast(I32), 0x7F800000, None,
                                op0=mybir.AluOpType.bitwise_and)
        mf = spool.tile([128, ngroups], F32, tag="wmf")
        nc.vector.tensor_scalar(mf[:], mi[:].bitcast(F32), MAGIC_MUL, None,
                                op0=mybir.AluOpType.mult)
        mb = mf[:].unsqueeze(-1).broadcast_to([128, ngroups, GS])
        nc.vector._custom_dve(snap, out=g3(out_tile[:]), in0=g3(src_ap), in1=mb)

    with tile.TileContext(nc) as tc:
        with ExitStack() as ctx:
            pool = ctx.enter_context(tc.tile_pool(name="big", bufs=3))
            spool = ctx.enter_context(tc.tile_pool(name="small", bufs=3))
            # weight first: its tiny DMA lands long before sample 0's slab
            wf = pool.tile([128, 288], F32, tag="wf")
            nc.sync.dma_start(wf[:], win[:].rearrange("o i h w -> (o i h w)")
                              .rearrange("(p c) -> p c", p=128))
            qwt = pool.tile([128, 288], BF16, tag="qwt")
            bfp_w(pool, spool, wf[:], 8, qwt)
            nc.scalar.dma_start(qw[:], qwt[:])
            xr = xin[:].rearrange("j p c -> p j c")
            qr = qx[:].rearrange("j p c -> p j c")
            for j in range(4):
                for h in range(2):
                    cols, off = CHUNK_COLS[h], CHUNK_OFF[h]
                    ng = cols // GS
                    g3 = lambda ap: ap.rearrange("p (g s) -> p g s", s=GS)
                    xs = pool.tile([128, cols], F32, tag=f"xs{h}")
                    nc.sync.dma_start(xs[:], xr[:, j, off:off + cols])
                    # scalar engine: cast to bf16 (feeds both reduce and snap)
                    xb = pool.tile([128, cols], BF16, tag=f"xb{h}")
                    nc.scalar.copy(xb[:], xs[:])
                    # vector: group abs-max on bf16 (2x mode)
                    m = spool.tile([128, ng], BF16, tag=f"m{h}")
                    nc.vector.tensor_reduce(
                        m[:], g3(xb[:]), axis=mybir.AxisListType.X,
                        op=mybir.AluOpType.max, apply_absolute_value=True)
                    # M = exp_bits(max) * 1.5*2^16  (tiny)
                    mi = spool.tile([128, ng], I16, tag=f"mi{h}")
                    nc.vector.tensor_scalar(
                        mi[:], m[:].bitcast(I16), 0x7F80, None,
                        op0=mybir.AluOpType.bitwise_and)
                    mf = spool.tile([128, ng], BF16, tag=f"mf{h}")
                    nc.vector.tensor_scalar(
                        mf[:], mi[:].bitcast(BF16), MAGIC_MUL, None,
                        op0=mybir.AluOpType.mult)
                    # vector: snap (x + M) - M in one custom-DVE pass
                    q = pool.tile([128, cols], BF16, tag=f"q{h}", bufs=6)
                    mb = mf[:].unsqueeze(-1).broadcast_to([128, ng, GS])
                    nc.vector._custom_dve(snap, out=g3(q[:]), in0=g3(xb[:]),
                                          in1=mb)
                    nc.scalar.dma_start(qr[:, j, off:off + cols], q[:])
    nc.compile()
    return nc


def build_conv():
    """v2: 4x 64x64 PE-array tiles. T0/T2 convolve sample A (row-chunks 2t,
    2t+1), T8/T10 sample B, all four concurrently; weights replicated on
    both SBUF partition halves. Two PSUM banks per round (A and B)."""
    nc = bacc.Bacc(None)
    qx4 = nc.declare_dram_parameter("qx4", [4, C, WP, WP], BF16, isOutput=False)
    wblk = nc.declare_dram_parameter("wblk", [128, 9 * 64], BF16, isOutput=False)
    bias2 = nc.declare_dram_parameter("bias2", [128], F32, isOutput=False)
    # chunk-major: [sample, 4-row chunk, c, r, w]; host transposes to NCHW.
    # Keeps each out-DMA one call with 128 contiguous 1792B descriptors.
    out = nc.declare_dram_parameter("out", [4, 2, C, 14, 4, W], F32,
                                    isOutput=True)

    with tile.TileContext(nc) as tc:
        with ExitStack() as ctx:
            consts = ctx.enter_context(tc.tile_pool(name="consts", bufs=1))
            xpool = ctx.enter_context(tc.tile_pool(name="x", bufs=2))
            opool = ctx.enter_context(tc.tile_pool(name="o", bufs=4))
            psum = ctx.enter_context(tc.tile_pool(name="ps", bufs=2, space="PSUM"))

            # wblk[p, tap*64 + o]: W[tap][ic=p%64, oc=o], same on both halves
            wsb = consts.tile([128, 9 * 64], BF16)
            nc.sync.dma_start(wsb[:], wblk[:])
            bias_sb = consts.tile([128, 1], F32)
            nc.sync.dma_start(bias_sb[:], bias2[:, None])

            for p in range(2):
                xpad = xpool.tile([128, XPAD], BF16, tag="xpad")
                nc.gpsimd.memset(xpad[:, 0:1], 0.0)           # guard slots
                nc.gpsimd.memset(xpad[:, XPAD - 1:XPAD], 0.0)
                # load in 4 row-bands so round 0 can start after band 0;
                # band k covers padded rows [29k, 29k+29) (+1 trailing row)
                for k in range(4):
                    rlo = 29 * k
                    nrows = 29 if k < 3 else 27
                    nc.sync.dma_start(
                        xpad[0:64, 1 + rlo * WP:1 + (rlo + nrows) * WP],
                        qx4[2 * p, :, rlo:rlo + nrows, :]
                        .rearrange("c h w -> c (h w)"))
                    nc.sync.dma_start(
                        xpad[64:128, 1 + rlo * WP:1 + (rlo + nrows) * WP],
                        qx4[2 * p + 1, :, rlo:rlo + nrows, :]
                        .rearrange("c h w -> c (h w)"))

                for t in range(14):
                    r0 = 8 * t
                    psA = psum.tile([128, 456], F32, tag="psA")
                    psB = psum.tile([128, 456], F32, tag="psB")
                    for tap in range(9):
                        dh, dw = divmod(tap, 3)
                        b0 = 1 + (r0 + dh) * WP + dw - 1
                        b1 = 1 + (r0 + 4 + dh) * WP + dw - 1
                        st, sp = (tap == 0), (tap == 8)
                        w_lo = wsb2[0:64, tap * 64:(tap + 1) * 64]
                        w_hi = wsb2[64:128, tap * 64:(tap + 1) * 64]
                        nc.tensor.matmul(psA[0:64, :], w_lo,
                                         xpad[0:64, b0:b0 + 456],
                                         start=st, stop=sp,
                                         tile_position=(0, 0))
                        nc.tensor.matmul(psB[0:64, :], w_hi,
                                         xpad[64:128, b0:b0 + 456],
                                         start=st, stop=sp,
                                         tile_position=(64, 0))
                        nc.tensor.matmul(psA[64:128, :], w_lo,
                                         xpad[0:64, b1:b1 + 456],
                                         start=st, stop=sp,
                                         tile_position=(0, 64))
                        nc.tensor.matmul(psB[64:128, :], w_hi,
                                         xpad[64:128, b1:b1 + 456],
                                         start=st, stop=sp,
                                         tile_position=(64, 64))
                    for s, ps in ((0, psA), (1, psB)):
                        # compact 114->112 in the evac so the DMA source is
                        # contiguous: one 1792B descriptor per partition
                        osb = opool.tile([128, 448], F32, tag=f"osb{s}")
                        nc.vector.tensor_scalar(
                            osb[:].rearrange("p (r w) -> p r w", w=W),
                            ps[:].rearrange("p (r w) -> p r w", w=WP)
                            [:, :, 1:113],
                            bias_sb[:, 0:1], None,
                            op0=mybir.AluOpType.add)
                        nc.scalar.dma_start(
                            out[2 * p + s, 2 * t:2 * t + 2]
                            .rearrange("u c r w -> (u c) (r w)"),
                            osb[:])
    nc.compile()
    return nc


def build_fused():
    """Single program: BFP-quantize x (f32 path), bounce q through a DRAM
    scratch tile, indirect-DMA it back in conv layout (per-channel dynamic
    offsets absorb the per-sample group phase), then the 4x64x64-tiled conv.
    Weights arrive pre-quantized from the host (exact numpy BFP)."""
    snap = _ensure_snap_op()
    nc = bacc.Bacc(None, num_swdge_queues=2)
    xin = nc.declare_dram_parameter("xin", [4, 128, QCOLS], F32, isOutput=False)
    wblk = nc.declare_dram_parameter("wblk", [128, 9 * 64], BF16, isOutput=False)
    bias2 = nc.declare_dram_parameter("bias2", [128], F32, isOutput=False)
    ofs = nc.declare_dram_parameter("ofs", [32, 8], I32, isOutput=False)
    out = nc.declare_dram_parameter("out", [4, 2, C, 14, 4, W], F32,
                                    isOutput=True)

    g3 = lambda ap: ap.rearrange("p (g s) -> p g s", s=GS)

    with tile.TileContext(nc) as tc:
        with ExitStack() as ctx:
            consts = ctx.enter_context(tc.tile_pool(name="consts", bufs=1))
            qpool = ctx.enter_context(tc.tile_pool(name="qd", bufs=1,
                                                   space="DRAM"))
            xpool = ctx.enter_context(tc.tile_pool(name="x", bufs=2))
            spool = ctx.enter_context(tc.tile_pool(name="small", bufs=3))
            xpadp = ctx.enter_context(tc.tile_pool(name="xp", bufs=2))
            opool = ctx.enter_context(tc.tile_pool(name="o", bufs=4))
            psum = ctx.enter_context(tc.tile_pool(name="ps", bufs=4,
                                                  space="PSUM"))

            wsb = consts.tile([128, 9 * 64], BF16)
            nc.sync.dma_start(wsb[:], wblk[:])
            bias_sb = consts.tile([128, 1], F32)
            nc.sync.dma_start(bias_sb[:], bias2[:, None])
            ofs_sb = consts.tile([32, 8], I32)
            nc.sync.dma_start(ofs_sb[:], ofs[:])

            qfs = [qpool.tile([128 * QCOLS, 1], BF16, tag=f"qf{j}",
                              name=f"qf{j}") for j in range(4)]

            def quant_sample(j):
                qlast = None
                for h in range(2):
                    cols, off = CHUNK_COLS[h], CHUNK_OFF[h]
                    ng = cols // GS
                    xs = xpool.tile([128, cols], F32, tag=f"xs{h}")
                    nc.sync.dma_start(xs[:],
                                      xin[:].rearrange("j p c -> p j c")
                                      [:, j, off:off + cols])
                    m = spool.tile([128, ng], F32, tag=f"m{h}")
                    nc.vector.tensor_reduce(
                        m[:], g3(xs[:]), axis=mybir.AxisListType.X,
                        op=mybir.AluOpType.max, apply_absolute_value=True)
                    mi = spool.tile([128, ng], I32, tag=f"mi{h}")
                    nc.vector.tensor_scalar(
                        mi[:], m[:].bitcast(I32), 0x7F800000, None,
                        op0=mybir.AluOpType.bitwise_and)
                    mf = spool.tile([128, ng], F32, tag=f"mf{h}")
                    nc.vector.tensor_scalar(
                        mf[:], mi[:].bitcast(F32), MAGIC_MUL, None,
                        op0=mybir.AluOpType.mult)
                    q = xpool.tile([128, cols], BF16, tag=f"q{h}", bufs=2)
                    mb = mf[:].unsqueeze(-1).broadcast_to([128, ng, GS])
                    nc.vector._custom_dve(snap, out=g3(q[:]), in0=g3(xs[:]),
                                          in1=mb)
                    nc.scalar.dma_start(
                        qfs[j][:].rearrange("(p c) one -> p (c one)", p=128)
                        [:, off:off + cols], q[:])
                    qlast = q
                return qlast

            def readback(j, xflat):
                half = j % 2
                pb = 64 * half
                if os.environ.get("FV_STATIC_RB"):
                    nc.sync.dma_start(
                        xflat[pb:pb + 64, :],
                        qfs[j][:].rearrange("n one -> (n one)")
                        [0:64 * 12544].rearrange("(c f) -> c f", f=12544))
                    return
                for piece in range(2):
                    c0 = 32 * piece
                    nc.gpsimd.indirect_dma_start(
                        out=xflat[pb + c0:pb + c0 + 32, :],
                        out_offset=None,
                        in_=qfs[j][:],
                        in_offset=bass.IndirectOffsetOnAxis(
                            ap=ofs_sb[0:32, 2 * j + piece:2 * j + piece + 1],
                            axis=0))

            # tap validity: out(r,w) += W[dh,dw] * x(r+dh-1, w+dw-1); taps
            # reaching outside the sample are simply dropped (zero padding)
            TAPS = [(1, 1)] + [(dh, dw) for dh in range(3) for dw in range(3)
                               if (dh, dw) != (1, 1)]
            if os.environ.get("FV_CENTER_ONLY"):
                TAPS = [(1, 1)] * 9

            def conv_pair(p, xflat, wsb2=None):
                xv = xflat[:].rearrange("p (r w) -> p r w", w=W)
                stg = [opool.tile([128, 14 * 448], F32, tag=f"stg{s}",
                                  bufs=1, name=f"stg{s}_{p}")
                       for s in range(2)]
                for t in range(14):
                    r0 = 8 * t
                    psA = psum.tile([128, 448], F32, tag="psA")
                    psB = psum.tile([128, 448], F32, tag="psB")
                    views = {id(psA): psA[:].rearrange("p (r w) -> p r w", w=W),
                             id(psB): psB[:].rearrange("p (r w) -> p r w", w=W)}
                    for n, (dh, dw) in enumerate(TAPS):
                        st, sp = (n == 0), (n == 8)
                        tap = dh * 3 + dw
                        w_lo = wsb2[0:64, tap * 64:(tap + 1) * 64]
                        w_hi = wsb2[64:128, tap * 64:(tap + 1) * 64]
                        wlo = max(0, 1 - dw)
                        whi = min(W, W + 1 - dw)
                        mw = wlo + dw - 1
                        for half, (ps, wq, kb, cq) in enumerate((
                                (psA, w_lo, 0, 0), (psB, w_hi, 64, 0),
                                (psA, w_lo, 0, 64), (psB, w_hi, 64, 64))):
                            R = r0 + 4 * (cq // 64)
                            ilo = max(0, 1 - dh - R)
                            ihi = min(4, H + 1 - dh - R)
                            if ilo >= ihi or wlo >= whi:
                                continue
                            rb = R + ilo + dh - 1
                            pv = views[id(ps)]
                            nc.tensor.matmul(
                                pv[cq:cq + 64, ilo:ihi, wlo:whi],
                                wq,
                                xv[kb:kb + 64, rb:rb + ihi - ilo, mw:mw + whi - wlo],
                                start=st, stop=sp,
                                tile_position=(kb, cq))
                    for s, ps in ((0, psA), (1, psB)):
                        nc.scalar.add(stg[s][:, 448 * t:448 * t + 448],
                                      ps[:], bias_sb[:, 0:1])
                for s in range(2):
                    for hh in range(4):
                        t0 = (0, 4, 8, 11)[hh]
                        t1 = (4, 8, 11, 14)[hh]
                        nc.scalar.dma_start(
                            out[2 * p + s, :, :, t0:t1]
                            .rearrange("u c t r w -> (u c) (t r w)"),
                            stg[s][:, 448 * t0:448 * t1])

            xflats = {}
            for p in range(2):
                xflats[p] = xpadp.tile([128, SAMPLE // 64], BF16, tag="xpad",
                                       name=f"xflat{p}")
                for j in (2 * p, 2 * p + 1):
                    qlast = quant_sample(j)
                    readback(j, xflats[p])
            # value-safe dependency: conv phase starts only after the last
            # readback, so the tensor engine runs with the DMA/vector quiet
            # (avoids the power-state penalty measured when phases overlap)
            tok = consts.tile([128, 1], F32, name="tok")
            nc.vector.tensor_tensor(tok[:], xflats[1][:, 0:1],
                                    xflats[1][:, 0:1],
                                    op=mybir.AluOpType.is_equal)
            wsb2 = consts.tile([128, 9 * 64], BF16, name="wsb2")
            nc.vector.tensor_scalar(wsb2[:], wsb[:], tok[:, 0:1], None,
                                    op0=mybir.AluOpType.mult)
            for p in range(2):
                conv_pair(p, xflats[p], wsb2)
    nc.compile()
    return nc


def _bfp_quantize_np(w):
    """Exact host-side BFP quantization (matches the jax reference bit-for-
    bit in f32: scale is a power of two, round is half-even)."""
    flat = np.asarray(w, np.float32).reshape(-1)
    n = flat.shape[0]
    pad = (-n) % GS
    f = np.pad(flat, (0, pad)).reshape(-1, GS)
    m = np.max(np.abs(f), axis=1, keepdims=True)
    mant, ex = np.frexp(np.where(m > 0, m, 1.0))   # m = mant*2^ex, mant in [.5,1)
    scale = np.ldexp(np.float32(1.0), ex - 1 - 7).astype(np.float32)
    q = np.round(f / scale) * scale
    q = np.where(m > 0, q, 0.0).astype(np.float32)
    return q.reshape(-1)[:n].reshape(np.asarray(w).shape)


def _shard_inputs(x, weight):
    """Build per-core in_maps for program A."""
    xf = np.ascontiguousarray(x, dtype=np.float32).reshape(-1)
    xf = np.concatenate([xf, np.zeros(QWIN, np.float32)])
    in_maps = []
    pres = []
    for k in range(N_CORES):
        core_pre = []
        xin = np.empty((4, 128, QCOLS), np.float32)
        for j in range(4):
            s = 4 * k + j
            start = s * SAMPLE
            gstart = (start // GS) * GS
            core_pre.append(start - gstart)
            xin[j] = xf[gstart:gstart + QWIN].reshape(128, QCOLS)
        in_maps.append({"xin": xin, "w": np.ascontiguousarray(weight, np.float32)})
        pres.append(core_pre)
    return in_maps, pres


def kernel(x, weight, bias):
    from concourse.bass_utils import run_bass_kernel_spmd

    if "fused" not in _cache:
        _cache["fused"] = build_fused()

    core_ids = list(range(N_CORES))
    trace = _trace_enabled()
    if trace:
        _install_trace_shim()

    in_maps, pres = _shard_inputs(x, weight)

    wq = _bfp_quantize_np(weight).reshape(64, 64, 9)    # [o,i,t] exact BFP
    wtio = wq.transpose(1, 2, 0).astype(ml_dtypes.bfloat16)  # [i,t,o]
    wblk = np.concatenate([wtio, wtio], axis=0).reshape(128, 9 * 64)
    bias2 = np.concatenate([np.asarray(bias, np.float32)] * 2)
    rvec = np.arange(32, dtype=np.int64) * (H * W)
    for k in range(N_CORES):
        ofs = np.empty((32, 8), np.int64)
        for j in range(4):
            for piece in range(2):
                ofs[:, 2 * j + piece] = (pres[k][j] + (32 * piece) * (H * W)
                                         + rvec)
        in_maps[k] = {"xin": in_maps[k]["xin"], "wblk": wblk,
                      "bias2": bias2, "ofs": ofs.astype(np.int32)}

    res = run_bass_kernel_spmd(_cache["fused"], in_maps, core_ids, trace=trace)
    last_exec_ns.clear()
    last_exec_ns["fused"] = res.exec_time_ns
    last_results["fused"] = res

    out = np.concatenate(
        [np.asarray(res.results[k]["out"]) for k in range(N_CORES)], axis=0)
    # [32, 2, C, 14, 4, W] -> NCHW (h = 8t + 4u + r)
    out = out.transpose(0, 2, 3, 1, 4, 5).reshape(B, C, H, W)
    return np.ascontiguousarray(out, dtype=np.float32)

